# revision 34
# baseline (speedup 1.0000x reference)
"""HRA (Householder Reflection Adaptation) forward kernel for Trainium2.

Math: out = x @ Q with Q = prod_i (I - 2 u_i u_i^T), u_i = normalized columns
of hra_u [4096, 8].  Using the compact WY representation:
    Q = I - U T U^T      (T upper-triangular 8x8, diag=2)
    out = x - (x @ A) @ U^T,   A = U @ T

Precision: the correctness gate is rel_err < 2e-2 against max|out| ~ 5.5;
bf16 end-to-end carries ~5e-3 max error, so the device works in bf16:
  - host casts x f32 -> bf16 (halves both HBM streams: 33.6 -> 16.8 MB/core)
  - device math is bf16 with f32 PSUM accumulation
  - device writes bf16; host casts the gathered result back to f32

Sharding: data-parallel over rows, 1024 rows/core, A/U^T replicated.

Per-core pipeline, 4 row-blocks of 256 rows (J=2 x 128):
  front(b): per 4-chunk group: 8 REGULAR matmuls x_c^T = x_c.T @ I
    (transpose-mode runs at a fixed ~219ns and never trips the HAM
    clock-gate; a regular matmul streams at the warm clock AND keeps the
    gate open) -> f32 PSUM [128,1024]; one ACT copy casts PSUM -> bf16 x^T
    in SBUF; then 4 proj matmuls accumulate P^T[40,256] (A is padded to 40
    cols with a duplicate at 32..39 so P^T lands at partition bases 0 AND
    32, feeding the row-packed update matmuls with no replication copies)
  back(b): per (j, 1024-col pair): two row-packed update matmuls (K=8 at
    array rows 0-7 / 32-39) -> f32 PSUM [128,2,512]; one contiguous DVE
    subtract (in-place into xb); DMA-out 512KB pieces on the ACT HWDGE
    ring (inputs ride the SP ring, split in halves so compute starts early)
  back(b-1) units interleave into front(b); transpose groups and update
  units share one 3-slot PSUM pool (their slots are both [128,1024] f32),
  leaving one bank spare beside the proj accumulator.
"""

import os
import sys

for _p in ("/opt/trn_rl_repo", "/root/.axon_site", "/root/.axon_site/_ro/trn_rl_repo",
           "/root/.axon_site/_ro/pypackages"):
    if os.path.isdir(_p) and _p not in sys.path:
        sys.path.append(_p)

import ml_dtypes
import numpy as np

import concourse.bass as bass
import concourse.mybir as mybir
import concourse.tile as tile
from concourse import bacc
from concourse.bass_utils import run_bass_kernel_spmd

B, S, D, R = 4, 2048, 4096, 8
N_CORES = 8
ROWS = B * S                      # 8192
RPC = ROWS // N_CORES             # 1024 rows per core
P = 128
J = 2                             # 128-row tiles per block
BLK = J * P                       # 256 rows per block
NB = RPC // BLK                   # 4 blocks per core
CH = D // P                       # 32 chunks of 128 cols
MPAD = 40                         # A padded to 40 cols (dup at 32..39)

F32 = mybir.dt.float32
BF16 = mybir.dt.bfloat16
NPBF16 = ml_dtypes.bfloat16

_CACHE = {}


def _householder_wy(hra_u: np.ndarray):
    """Return (A_pad [D,40], UT_pad [40,D]) with out = x - (x @ A) @ UT.

    Both carry a duplicate copy at rows/cols 32..39: the row-packed update
    matmuls need weight and fmap at the same partition base (0 and 32)."""
    u = hra_u.astype(np.float64)
    u = u / np.linalg.norm(u, axis=0, keepdims=True)
    T = np.zeros((R, R), np.float64)
    for k in range(R):
        T[k, k] = 2.0
        if k:
            T[:k, k] = -2.0 * (T[:k, :k] @ (u[:, :k].T @ u[:, k]))
    A = u @ T                                    # [D, R]
    A_pad = np.zeros((D, MPAD), np.float64)
    A_pad[:, :R] = A
    A_pad[:, 32:32 + R] = A
    UT_pad = np.zeros((MPAD, D), np.float64)
    UT_pad[:R] = u.T
    UT_pad[32:32 + R] = u.T
    return A_pad, np.ascontiguousarray(UT_pad)


def _build_program():
    nc = bacc.Bacc(trn_type="TRN2")
    x = nc.dram_tensor("x", (RPC, D), BF16, kind="ExternalInput")
    a = nc.dram_tensor("a", (P, CH * MPAD), BF16, kind="ExternalInput")
    ut = nc.dram_tensor("ut", (MPAD, D), BF16, kind="ExternalInput")
    ident = nc.dram_tensor("ident", (P, P), BF16, kind="ExternalInput")
    out = nc.dram_tensor("out", (RPC, D), BF16, kind="ExternalOutput")

    xd = x.rearrange("(b j p) d -> b p j d", p=P, j=J)
    od = out.rearrange("(b j p) d -> b p j d", p=P, j=J)
    H = D // 2

    with tile.TileContext(nc) as tc:
        with (
            tc.tile_pool(name="const", bufs=1) as const,
            tc.tile_pool(name="xp", bufs=2) as x_pool,
            tc.tile_pool(name="xtp", bufs=2) as xt_pool,
            tc.tile_pool(name="ptp", bufs=2) as pt_pool,
            tc.tile_pool(name="crp", bufs=2) as corr_pool,
            tc.tile_pool(name="pst", bufs=3, space="PSUM") as pst_pool,
            tc.tile_pool(name="psu", bufs=2, space="PSUM") as psu_pool,
            tc.tile_pool(name="psp", bufs=1, space="PSUM") as psp_pool,
        ):
            # identity first (the warm-up burst only needs it), then block-0
            # quarter 0 so the first transposes start ASAP, then the rest
            Q = D // 4
            ident_sb = const.tile([P, P], BF16)
            nc.sync.dma_start(ident_sb, ident[:, :])
            xbs = []
            xb0 = x_pool.tile([P, J, D], BF16, tag="xb", bufs=NB)
            xbs.append(xb0)
            for j in range(J):
                nc.sync.dma_start(xb0[:, j, :Q], xd[0, :, j, :Q])
            a_sb = const.tile([P, CH * MPAD], BF16)
            nc.sync.dma_start(a_sb, a[:, :])
            for q in range(1, 4):
                for j in range(J):
                    nc.sync.dma_start(xb0[:, j, q * Q:(q + 1) * Q],
                                      xd[0, :, j, q * Q:(q + 1) * Q])
            ut_sb = const.tile([MPAD, D], BF16)
            nc.sync.dma_start(ut_sb, ut[:, :])

            # A ~5us matmul burst during the DMA fill opens the HAM gate
            # before the first real transposes; then prime PE on the other
            # constants (one sync-wait per LDWEIGHTS).
            warm = psu_pool.tile([P, 2, 512], F32, tag="ps_u")
            warm_t = pst_pool.tile([P, 4, BLK], BF16, tag="ps_t")
            nc.tensor.matmul(warm[:P, 0, :P], ident_sb, ident_sb,
                             start=True, stop=True)
            for _ in range(56):
                nc.tensor.matmul(warm[:, 1, :P], ident_sb, ident_sb,
                                 start=True, stop=True)
            nc.tensor.transpose(warm_t[:, 0, :P], ident_sb, ident_sb)
            nc.tensor.matmul(warm[:MPAD, 0, :P], a_sb[:, :MPAD], a_sb[:, :P],
                             start=True, stop=True)
            nc.tensor.matmul(warm[:, 0, :], ut_sb[:, :P], ut_sb[:, :512],
                             start=True, stop=True)

            # prefetch remaining block inputs: one 2MB dma_start per block
            # (two contiguous 8KB lines per partition) - fewer, larger DMAs
            # keep the SP issue queue short and the SDMA engines saturated
            for b in range(1, NB):
                xb = x_pool.tile([P, J, D], BF16, tag="xb", bufs=NB)
                xbs.append(xb)
                nc.sync.dma_start(xb, xd[b])

            pts = {}

            def front_units(b):
                """yield per-group callables: 8 transpose matmuls -> bf16
                PSUM, ACT copy -> bf16 x^T.  Each group's proj matmuls are
                delayed by one group so they never stall the in-order PE
                queue waiting on the ACT copy."""
                proj_ps = psp_pool.tile([MPAD, BLK], F32, tag="ps_p")

                def proj_burst(g):
                    for cl in range(4):
                        c = 4 * g + cl
                        nc.tensor.matmul(
                            proj_ps,
                            a_sb[:, c * MPAD:(c + 1) * MPAD],
                            xts[b][:, c, :],
                            start=(c == 0),
                            stop=(c == CH - 1),
                        )

                def group(g):
                    if g > 0:
                        proj_burst(g - 1)
                    ps_t = pst_pool.tile([P, 4, BLK], BF16, tag="ps_t")
                    for cl in range(4):
                        c = 4 * g + cl
                        for j in range(J):
                            nc.tensor.transpose(
                                ps_t[:, cl, j * P:(j + 1) * P],
                                xbs[b][:, j, c * P:(c + 1) * P],
                                ident_sb,
                            )
                    nc.scalar.copy(xts[b][:, 4 * g:4 * g + 4, :].bitcast(F32),
                                   ps_t.bitcast(F32))

                def finish():
                    proj_burst(7)
                    pt = pt_pool.tile([MPAD, BLK], BF16, tag="pt")
                    nc.vector.tensor_copy(pt, proj_ps)
                    pts[b] = pt

                for g in range(8):
                    yield lambda g=g: group(g)
                yield lambda: finish()

            def back_units(b):
                """yield per-(j, col-pair) row-packed update + subtract"""
                pt = pts[b]
                xb = xbs[b]

                def unit(j, dp):
                    lo, hi = dp * 1024, (dp + 1) * 1024
                    ps_u = psu_pool.tile([P, 2, 512], F32, tag="ps_u")
                    nc.tensor.matmul(
                        ps_u[:, 0, :],
                        pt[0:R, j * P:(j + 1) * P],
                        ut_sb[0:R, lo:lo + 512],
                        start=True, stop=True,
                        tile_position=(0, 0),
                    )
                    nc.tensor.matmul(
                        ps_u[:, 1, :],
                        pt[32:32 + R, j * P:(j + 1) * P],
                        ut_sb[32:32 + R, lo + 512:hi],
                        start=True, stop=True,
                        tile_position=(32, 0),
                    )
                    corr = ps_u.rearrange("p a n -> p (a n)")
                    if dp % 2 == 1:
                        # route the correction through an ACT copy to SBUF
                        # bf16: the follow-up all-bf16 DVE subtract runs at
                        # 2x (a PSUM-f32 operand pins tensor_sub at 1x), so
                        # half the serial DVE chain moves onto ACT slack
                        corr_sb = corr_pool.tile([P, 1024], BF16, tag="corr")
                        nc.scalar.copy(corr_sb, corr)
                        corr = corr_sb
                    nc.vector.tensor_sub(
                        xb[:, j, lo:hi],
                        xb[:, j, lo:hi],
                        corr,
                    )
                    if dp % 2 == 1:
                        nc.scalar.dma_start(
                            od[b, :, j, lo - 1024:hi],
                            xb[:, j, lo - 1024:hi],
                        )

                for j in range(J):
                    for dp in range(4):
                        yield lambda j=j, dp=dp: unit(j, dp)

            def drain(it):
                for f in it:
                    f()

            xts = [xt_pool.tile([P, CH, BLK], BF16, tag="xt", name=f"xt{b}")
                   for b in range(NB)]

            drain(front_units(0))
            for b in range(1, NB):
                fu = list(front_units(b))
                bu = list(back_units(b - 1))
                order = []
                for i, f in enumerate(fu):
                    if i < len(bu):
                        order.append(bu[i])
                    order.append(f)
                drain(order)
            drain(back_units(NB - 1))

    nc.compile()
    return nc


def _get_program():
    if "nc" not in _CACHE:
        _CACHE["nc"] = _build_program()
    return _CACHE["nc"]


def kernel(input, hra_u, **run_kwargs):
    input = np.asarray(input, dtype=np.float32)
    hra_u = np.asarray(hra_u, dtype=np.float32)

    A_pad, UT = _householder_wy(hra_u)
    # pack A_pad [D, 40] so partition p holds A_pad[c*128+p, :] at offset c*40
    a_packed = np.ascontiguousarray(
        A_pad.reshape(CH, P, MPAD).transpose(1, 0, 2).reshape(P, CH * MPAD)
    ).astype(NPBF16)
    ut_b = UT.astype(NPBF16)
    ident = np.eye(P, dtype=NPBF16)

    x_flat = input.reshape(ROWS, D).astype(NPBF16)
    in_maps = [
        {
            "x": x_flat[c * RPC:(c + 1) * RPC],
            "a": a_packed,
            "ut": ut_b,
            "ident": ident,
        }
        for c in range(N_CORES)
    ]

    nc = _get_program()
    res = run_bass_kernel_spmd(nc, in_maps, core_ids=list(range(N_CORES)),
                               **run_kwargs)
    out = np.concatenate([r["out"] for r in res.results], axis=0)
    if run_kwargs:
        kernel.last_results = res
    return out.astype(np.float32).reshape(B, S, D)


# revision 35
# speedup vs baseline: 1.0539x; 1.0539x over previous
"""HRA (Householder Reflection Adaptation) forward kernel for Trainium2.

Math: out = x @ Q with Q = prod_i (I - 2 u_i u_i^T), u_i = normalized columns
of hra_u [4096, 8].  Using the compact WY representation:
    Q = I - U T U^T      (T upper-triangular 8x8, diag=2)
    out = x - (x @ A) @ U^T,   A = U @ T

Precision: the correctness gate is rel_err < 2e-2 against max|out| ~ 5.5;
bf16 end-to-end carries ~5e-3 max error, so the device works in bf16:
  - host casts x f32 -> bf16 (halves both HBM streams: 33.6 -> 16.8 MB/core)
  - device math is bf16 with f32 PSUM accumulation
  - device writes bf16; host casts the gathered result back to f32

Sharding: data-parallel over rows, 1024 rows/core, A/U^T replicated.

Per-core pipeline, 4 row-blocks of 256 rows (J=2 x 128):
  front(b): per 4-chunk group: 8 REGULAR matmuls x_c^T = x_c.T @ I
    (transpose-mode runs at a fixed ~219ns and never trips the HAM
    clock-gate; a regular matmul streams at the warm clock AND keeps the
    gate open) -> f32 PSUM [128,1024]; one ACT copy casts PSUM -> bf16 x^T
    in SBUF; then 4 proj matmuls accumulate P^T[40,256] (A is padded to 40
    cols with a duplicate at 32..39 so P^T lands at partition bases 0 AND
    32, feeding the row-packed update matmuls with no replication copies)
  back(b): per (j, 1024-col pair): two row-packed update matmuls (K=8 at
    array rows 0-7 / 32-39) -> f32 PSUM [128,2,512]; one contiguous DVE
    subtract (in-place into xb); DMA-out 512KB pieces on the ACT HWDGE
    ring (inputs ride the SP ring, split in halves so compute starts early)
  back(b-1) units interleave into front(b); transpose groups and update
  units share one 3-slot PSUM pool (their slots are both [128,1024] f32),
  leaving one bank spare beside the proj accumulator.
"""

import os
import sys

for _p in ("/opt/trn_rl_repo", "/root/.axon_site", "/root/.axon_site/_ro/trn_rl_repo",
           "/root/.axon_site/_ro/pypackages"):
    if os.path.isdir(_p) and _p not in sys.path:
        sys.path.append(_p)

import ml_dtypes
import numpy as np

import concourse.bass as bass
import concourse.mybir as mybir
import concourse.tile as tile
from concourse import bacc
from concourse.bass_utils import run_bass_kernel_spmd

B, S, D, R = 4, 2048, 4096, 8
N_CORES = 8
ROWS = B * S                      # 8192
RPC = ROWS // N_CORES             # 1024 rows per core
P = 128
J = 2                             # 128-row tiles per block
BLK = J * P                       # 256 rows per block
NB = RPC // BLK                   # 4 blocks per core
CH = D // P                       # 32 chunks of 128 cols
MPAD = 40                         # A padded to 40 cols (dup at 32..39)

F32 = mybir.dt.float32
BF16 = mybir.dt.bfloat16
NPBF16 = ml_dtypes.bfloat16

_CACHE = {}


def _householder_wy(hra_u: np.ndarray):
    """Return (A_pad [D,40], UT_pad [40,D]) with out = x - (x @ A) @ UT.

    Both carry a duplicate copy at rows/cols 32..39: the row-packed update
    matmuls need weight and fmap at the same partition base (0 and 32)."""
    u = hra_u.astype(np.float64)
    u = u / np.linalg.norm(u, axis=0, keepdims=True)
    T = np.zeros((R, R), np.float64)
    for k in range(R):
        T[k, k] = 2.0
        if k:
            T[:k, k] = -2.0 * (T[:k, :k] @ (u[:, :k].T @ u[:, k]))
    A = u @ T                                    # [D, R]
    A_pad = np.zeros((D, MPAD), np.float64)
    A_pad[:, :R] = A
    A_pad[:, 32:32 + R] = A
    UT_pad = np.zeros((MPAD, D), np.float64)
    UT_pad[:R] = u.T
    UT_pad[32:32 + R] = u.T
    return A_pad, np.ascontiguousarray(UT_pad)


def _build_program():
    nc = bacc.Bacc(trn_type="TRN2")
    x = nc.dram_tensor("x", (RPC, D), BF16, kind="ExternalInput")
    a = nc.dram_tensor("a", (P, CH * MPAD), BF16, kind="ExternalInput")
    ut = nc.dram_tensor("ut", (MPAD, D), BF16, kind="ExternalInput")
    ident = nc.dram_tensor("ident", (P, P), BF16, kind="ExternalInput")
    out = nc.dram_tensor("out", (RPC, D), BF16, kind="ExternalOutput")

    xd = x.rearrange("(b j p) d -> b p j d", p=P, j=J)
    od = out.rearrange("(b j p) d -> b p j d", p=P, j=J)
    H = D // 2

    with tile.TileContext(nc) as tc:
        with (
            tc.tile_pool(name="const", bufs=1) as const,
            tc.tile_pool(name="xp", bufs=2) as x_pool,
            tc.tile_pool(name="xtp", bufs=2) as xt_pool,
            tc.tile_pool(name="ptp", bufs=2) as pt_pool,
            tc.tile_pool(name="crp", bufs=2) as corr_pool,
            tc.tile_pool(name="pst", bufs=3, space="PSUM") as pst_pool,
            tc.tile_pool(name="psu", bufs=2, space="PSUM") as psu_pool,
            tc.tile_pool(name="psp", bufs=1, space="PSUM") as psp_pool,
        ):
            # identity first (the warm-up burst only needs it), then block-0
            # quarter 0 so the first transposes start ASAP, then the rest
            Q = D // 4
            ident_sb = const.tile([P, P], BF16)
            nc.sync.dma_start(ident_sb, ident[:, :])
            xbs = []
            xb0 = x_pool.tile([P, J, D], BF16, tag="xb", bufs=NB)
            xbs.append(xb0)
            for j in range(J):
                nc.sync.dma_start(xb0[:, j, :Q], xd[0, :, j, :Q])
            a_sb = const.tile([P, CH * MPAD], BF16)
            nc.sync.dma_start(a_sb, a[:, :])
            for q in range(1, 4):
                for j in range(J):
                    nc.sync.dma_start(xb0[:, j, q * Q:(q + 1) * Q],
                                      xd[0, :, j, q * Q:(q + 1) * Q])
            ut_sb = const.tile([MPAD, D], BF16)
            nc.sync.dma_start(ut_sb, ut[:, :])

            # A ~5us matmul burst during the DMA fill opens the HAM gate
            # before the first real transposes; then prime PE on the other
            # constants (one sync-wait per LDWEIGHTS).
            warm = psu_pool.tile([P, 2, 512], F32, tag="ps_u")
            warm_t = pst_pool.tile([P, 4, BLK], BF16, tag="ps_t")
            nc.tensor.matmul(warm[:P, 0, :P], ident_sb, ident_sb,
                             start=True, stop=True)
            for _ in range(56):
                nc.tensor.matmul(warm[:, 1, :P], ident_sb, ident_sb,
                                 start=True, stop=True)
            nc.tensor.transpose(warm_t[:, 0, :P], ident_sb, ident_sb)
            nc.tensor.matmul(warm[:MPAD, 0, :P], a_sb[:, :MPAD], a_sb[:, :P],
                             start=True, stop=True)
            nc.tensor.matmul(warm[:, 0, :], ut_sb[:, :P], ut_sb[:, :512],
                             start=True, stop=True)

            # prefetch remaining block inputs: one 2MB dma_start per block
            # (two contiguous 8KB lines per partition) - fewer, larger DMAs
            # keep the SP issue queue short and the SDMA engines saturated
            for b in range(1, NB):
                xb = x_pool.tile([P, J, D], BF16, tag="xb", bufs=NB)
                xbs.append(xb)
                nc.sync.dma_start(xb, xd[b])

            pts = {}

            def front_units(b):
                """yield per-group callables: 8 transpose matmuls -> bf16
                PSUM, ACT copy -> bf16 x^T.  Each group's proj matmuls are
                delayed by one group so they never stall the in-order PE
                queue waiting on the ACT copy."""
                proj_ps = psp_pool.tile([MPAD, BLK], F32, tag="ps_p")

                def proj_burst(g):
                    for cl in range(4):
                        c = 4 * g + cl
                        nc.tensor.matmul(
                            proj_ps,
                            a_sb[:, c * MPAD:(c + 1) * MPAD],
                            xts[b][:, c, :],
                            start=(c == 0),
                            stop=(c == CH - 1),
                        )

                def group(g):
                    if g > 0:
                        proj_burst(g - 1)
                    ps_t = pst_pool.tile([P, 4, BLK], BF16, tag="ps_t")
                    for cl in range(4):
                        c = 4 * g + cl
                        for j in range(J):
                            nc.tensor.transpose(
                                ps_t[:, cl, j * P:(j + 1) * P],
                                xbs[b][:, j, c * P:(c + 1) * P],
                                ident_sb,
                            )
                    nc.scalar.copy(xts[b][:, 4 * g:4 * g + 4, :].bitcast(F32),
                                   ps_t.bitcast(F32))

                def finish():
                    proj_burst(7)
                    pt = pt_pool.tile([MPAD, BLK], BF16, tag="pt")
                    nc.vector.tensor_copy(pt, proj_ps)
                    pts[b] = pt

                for g in range(8):
                    yield lambda g=g: group(g)
                yield lambda: finish()

            def back_units(b):
                """yield per-(j, col-pair) row-packed update + subtract"""
                pt = pts[b]
                xb = xbs[b]

                def unit(j, dp):
                    lo, hi = dp * 1024, (dp + 1) * 1024
                    ps_u = psu_pool.tile([P, 2, 512], F32, tag="ps_u")
                    nc.tensor.matmul(
                        ps_u[:, 0, :],
                        pt[0:R, j * P:(j + 1) * P],
                        ut_sb[0:R, lo:lo + 512],
                        start=True, stop=True,
                        tile_position=(0, 0),
                    )
                    nc.tensor.matmul(
                        ps_u[:, 1, :],
                        pt[32:32 + R, j * P:(j + 1) * P],
                        ut_sb[32:32 + R, lo + 512:hi],
                        start=True, stop=True,
                        tile_position=(32, 0),
                    )
                    corr = ps_u.rearrange("p a n -> p (a n)")
                    if dp % 2 == 0:
                        # route the correction through an ACT copy to SBUF
                        # bf16: the follow-up all-bf16 DVE subtract runs at
                        # 2x (a PSUM-f32 operand pins tensor_sub at 1x), so
                        # half the serial DVE chain moves onto ACT slack
                        corr_sb = corr_pool.tile([P, 1024], BF16, tag="corr")
                        nc.scalar.copy(corr_sb, corr)
                        corr = corr_sb
                    nc.vector.tensor_sub(
                        xb[:, j, lo:hi],
                        xb[:, j, lo:hi],
                        corr,
                    )
                    if dp % 2 == 1:
                        nc.scalar.dma_start(
                            od[b, :, j, lo - 1024:hi],
                            xb[:, j, lo - 1024:hi],
                        )

                for j in range(J):
                    for dp in range(4):
                        yield lambda j=j, dp=dp: unit(j, dp)

            def drain(it):
                for f in it:
                    f()

            xts = [xt_pool.tile([P, CH, BLK], BF16, tag="xt", name=f"xt{b}")
                   for b in range(NB)]

            drain(front_units(0))
            for b in range(1, NB):
                fu = list(front_units(b))
                bu = list(back_units(b - 1))
                order = []
                for i, f in enumerate(fu):
                    if i < len(bu):
                        order.append(bu[i])
                    order.append(f)
                drain(order)
            drain(back_units(NB - 1))

    nc.compile()
    return nc


def _get_program():
    if "nc" not in _CACHE:
        _CACHE["nc"] = _build_program()
    return _CACHE["nc"]


def kernel(input, hra_u, **run_kwargs):
    input = np.asarray(input, dtype=np.float32)
    hra_u = np.asarray(hra_u, dtype=np.float32)

    A_pad, UT = _householder_wy(hra_u)
    # pack A_pad [D, 40] so partition p holds A_pad[c*128+p, :] at offset c*40
    a_packed = np.ascontiguousarray(
        A_pad.reshape(CH, P, MPAD).transpose(1, 0, 2).reshape(P, CH * MPAD)
    ).astype(NPBF16)
    ut_b = UT.astype(NPBF16)
    ident = np.eye(P, dtype=NPBF16)

    x_flat = input.reshape(ROWS, D).astype(NPBF16)
    in_maps = [
        {
            "x": x_flat[c * RPC:(c + 1) * RPC],
            "a": a_packed,
            "ut": ut_b,
            "ident": ident,
        }
        for c in range(N_CORES)
    ]

    nc = _get_program()
    res = run_bass_kernel_spmd(nc, in_maps, core_ids=list(range(N_CORES)),
                               **run_kwargs)
    out = np.concatenate([r["out"] for r in res.results], axis=0)
    if run_kwargs:
        kernel.last_results = res
    return out.astype(np.float32).reshape(B, S, D)


# revision 36
# speedup vs baseline: 1.1002x; 1.0439x over previous
"""HRA (Householder Reflection Adaptation) forward kernel for Trainium2.

Math: out = x @ Q with Q = prod_i (I - 2 u_i u_i^T), u_i = normalized columns
of hra_u [4096, 8].  Using the compact WY representation:
    Q = I - U T U^T      (T upper-triangular 8x8, diag=2)
    out = x - (x @ A) @ U^T,   A = U @ T

Precision: the correctness gate is rel_err < 2e-2 against max|out| ~ 5.5;
bf16 end-to-end carries ~5e-3 max error, so the device works in bf16:
  - host casts x f32 -> bf16 (halves both HBM streams: 33.6 -> 16.8 MB/core)
  - device math is bf16 with f32 PSUM accumulation
  - device writes bf16; host casts the gathered result back to f32

Sharding: data-parallel over rows, 1024 rows/core, A/U^T replicated.

Per-core pipeline, 4 row-blocks of 256 rows (J=2 x 128):
  front(b): per 4-chunk group: 8 REGULAR matmuls x_c^T = x_c.T @ I
    (transpose-mode runs at a fixed ~219ns and never trips the HAM
    clock-gate; a regular matmul streams at the warm clock AND keeps the
    gate open) -> f32 PSUM [128,1024]; one ACT copy casts PSUM -> bf16 x^T
    in SBUF; then 4 proj matmuls accumulate P^T[40,256] (A is padded to 40
    cols with a duplicate at 32..39 so P^T lands at partition bases 0 AND
    32, feeding the row-packed update matmuls with no replication copies)
  back(b): per (j, 1024-col pair): two row-packed update matmuls (K=8 at
    array rows 0-7 / 32-39) -> f32 PSUM [128,2,512]; one contiguous DVE
    subtract (in-place into xb); DMA-out 512KB pieces on the ACT HWDGE
    ring (inputs ride the SP ring, split in halves so compute starts early)
  back(b-1) units interleave into front(b); transpose groups and update
  units share one 3-slot PSUM pool (their slots are both [128,1024] f32),
  leaving one bank spare beside the proj accumulator.
"""

import os
import sys

for _p in ("/opt/trn_rl_repo", "/root/.axon_site", "/root/.axon_site/_ro/trn_rl_repo",
           "/root/.axon_site/_ro/pypackages"):
    if os.path.isdir(_p) and _p not in sys.path:
        sys.path.append(_p)

import ml_dtypes
import numpy as np

import concourse.bass as bass
import concourse.mybir as mybir
import concourse.tile as tile
from concourse import bacc
from concourse.bass_utils import run_bass_kernel_spmd

B, S, D, R = 4, 2048, 4096, 8
N_CORES = 8
ROWS = B * S                      # 8192
RPC = ROWS // N_CORES             # 1024 rows per core
P = 128
J = 2                             # 128-row tiles per block
BLK = J * P                       # 256 rows per block
NB = RPC // BLK                   # 4 blocks per core
CH = D // P                       # 32 chunks of 128 cols
MPAD = 40                         # A padded to 40 cols (dup at 32..39)

F32 = mybir.dt.float32
BF16 = mybir.dt.bfloat16
NPBF16 = ml_dtypes.bfloat16

_CACHE = {}


def _householder_wy(hra_u: np.ndarray):
    """Return (A_pad [D,40], UT_pad [40,D]) with out = x - (x @ A) @ UT.

    Both carry a duplicate copy at rows/cols 32..39: the row-packed update
    matmuls need weight and fmap at the same partition base (0 and 32)."""
    u = hra_u.astype(np.float64)
    u = u / np.linalg.norm(u, axis=0, keepdims=True)
    T = np.zeros((R, R), np.float64)
    for k in range(R):
        T[k, k] = 2.0
        if k:
            T[:k, k] = -2.0 * (T[:k, :k] @ (u[:, :k].T @ u[:, k]))
    A = u @ T                                    # [D, R]
    A_pad = np.zeros((D, MPAD), np.float64)
    A_pad[:, :R] = A
    A_pad[:, 32:32 + R] = A
    UT_pad = np.zeros((MPAD, D), np.float64)
    UT_pad[:R] = u.T
    UT_pad[32:32 + R] = u.T
    return A_pad, np.ascontiguousarray(UT_pad)


def _build_program():
    nc = bacc.Bacc(trn_type="TRN2")
    x = nc.dram_tensor("x", (RPC, D), BF16, kind="ExternalInput")
    a = nc.dram_tensor("a", (P, CH * MPAD), BF16, kind="ExternalInput")
    ut = nc.dram_tensor("ut", (MPAD, D), BF16, kind="ExternalInput")
    ident = nc.dram_tensor("ident", (P, P), BF16, kind="ExternalInput")
    out = nc.dram_tensor("out", (RPC, D), BF16, kind="ExternalOutput")

    xd = x.rearrange("(b j p) d -> b p j d", p=P, j=J)
    od = out.rearrange("(b j p) d -> b p j d", p=P, j=J)
    H = D // 2

    with tile.TileContext(nc) as tc:
        with (
            tc.tile_pool(name="const", bufs=1) as const,
            tc.tile_pool(name="xp", bufs=2) as x_pool,
            tc.tile_pool(name="xtp", bufs=2) as xt_pool,
            tc.tile_pool(name="ptp", bufs=2) as pt_pool,
            tc.tile_pool(name="crp", bufs=2) as corr_pool,
            tc.tile_pool(name="pst", bufs=3, space="PSUM") as pst_pool,
            tc.tile_pool(name="psu", bufs=2, space="PSUM") as psu_pool,
            tc.tile_pool(name="psp", bufs=1, space="PSUM") as psp_pool,
        ):
            # identity first (the warm-up burst only needs it), then block-0
            # quarter 0 so the first transposes start ASAP, then the rest
            Q = D // 4
            ident_sb = const.tile([P, P], BF16)
            nc.sync.dma_start(ident_sb, ident[:, :])
            xbs = []
            xb0 = x_pool.tile([P, J, D], BF16, tag="xb", bufs=NB)
            xbs.append(xb0)
            for j in range(J):
                nc.sync.dma_start(xb0[:, j, :Q], xd[0, :, j, :Q])
            a_sb = const.tile([P, CH * MPAD], BF16)
            nc.sync.dma_start(a_sb, a[:, :])
            ut_sb = const.tile([MPAD, D], BF16)
            nc.sync.dma_start(ut_sb, ut[:, :])
            for q in range(1, 4):
                for j in range(J):
                    nc.sync.dma_start(xb0[:, j, q * Q:(q + 1) * Q],
                                      xd[0, :, j, q * Q:(q + 1) * Q])

            # A ~5us matmul burst during the DMA fill opens the HAM gate
            # before the first real transposes; then prime PE on the other
            # constants (one sync-wait per LDWEIGHTS).
            warm = psu_pool.tile([P, 2, 512], F32, tag="ps_u")
            warm_t = pst_pool.tile([P, 4, BLK], BF16, tag="ps_t")
            nc.tensor.matmul(warm[:P, 0, :P], ident_sb, ident_sb,
                             start=True, stop=True)
            for _ in range(56):
                nc.tensor.matmul(warm[:, 1, :P], ident_sb, ident_sb,
                                 start=True, stop=True)
            nc.tensor.transpose(warm_t[:, 0, :P], ident_sb, ident_sb)
            nc.tensor.matmul(warm[:MPAD, 0, :P], a_sb[:, :MPAD], a_sb[:, :P],
                             start=True, stop=True)
            nc.tensor.matmul(warm[:, 0, :], ut_sb[:, :P], ut_sb[:, :512],
                             start=True, stop=True)

            # prefetch remaining block inputs (in halves, block-major)
            for b in range(1, NB):
                xb = x_pool.tile([P, J, D], BF16, tag="xb", bufs=NB)
                xbs.append(xb)
                for h in range(2):
                    for j in range(J):
                        nc.sync.dma_start(xb[:, j, h * H:(h + 1) * H],
                                          xd[b, :, j, h * H:(h + 1) * H])

            pts = {}

            def front_units(b):
                """yield per-group callables: 8 transpose matmuls -> bf16
                PSUM, ACT copy -> bf16 x^T.  Each group's proj matmuls are
                delayed by one group so they never stall the in-order PE
                queue waiting on the ACT copy."""
                proj_ps = psp_pool.tile([MPAD, BLK], F32, tag="ps_p")

                def proj_burst(g):
                    for cl in range(4):
                        c = 4 * g + cl
                        nc.tensor.matmul(
                            proj_ps,
                            a_sb[:, c * MPAD:(c + 1) * MPAD],
                            xts[b][:, c, :],
                            start=(c == 0),
                            stop=(c == CH - 1),
                        )

                def group(g):
                    if g > 0:
                        proj_burst(g - 1)
                    ps_t = pst_pool.tile([P, 4, BLK], BF16, tag="ps_t")
                    for cl in range(4):
                        c = 4 * g + cl
                        for j in range(J):
                            nc.tensor.transpose(
                                ps_t[:, cl, j * P:(j + 1) * P],
                                xbs[b][:, j, c * P:(c + 1) * P],
                                ident_sb,
                            )
                    nc.scalar.copy(xts[b][:, 4 * g:4 * g + 4, :].bitcast(F32),
                                   ps_t.bitcast(F32))

                def finish():
                    proj_burst(7)
                    pt = pt_pool.tile([MPAD, BLK], BF16, tag="pt")
                    nc.vector.tensor_copy(pt, proj_ps)
                    pts[b] = pt

                for g in range(8):
                    yield lambda g=g: group(g)
                yield lambda: finish()

            def back_units(b):
                """yield per-(j, col-pair) row-packed update + subtract"""
                pt = pts[b]
                xb = xbs[b]

                def unit(j, dp):
                    lo, hi = dp * 1024, (dp + 1) * 1024
                    ps_u = psu_pool.tile([P, 2, 512], F32, tag="ps_u")
                    nc.tensor.matmul(
                        ps_u[:, 0, :],
                        pt[0:R, j * P:(j + 1) * P],
                        ut_sb[0:R, lo:lo + 512],
                        start=True, stop=True,
                        tile_position=(0, 0),
                    )
                    nc.tensor.matmul(
                        ps_u[:, 1, :],
                        pt[32:32 + R, j * P:(j + 1) * P],
                        ut_sb[32:32 + R, lo + 512:hi],
                        start=True, stop=True,
                        tile_position=(32, 0),
                    )
                    corr = ps_u.rearrange("p a n -> p (a n)")
                    if dp % 2 == 0:
                        # route the correction through an ACT copy to SBUF
                        # bf16: the follow-up all-bf16 DVE subtract runs at
                        # 2x (a PSUM-f32 operand pins tensor_sub at 1x), so
                        # half the serial DVE chain moves onto ACT slack
                        corr_sb = corr_pool.tile([P, 1024], BF16, tag="corr")
                        nc.scalar.copy(corr_sb, corr)
                        corr = corr_sb
                    nc.vector.tensor_sub(
                        xb[:, j, lo:hi],
                        xb[:, j, lo:hi],
                        corr,
                    )
                    if dp % 2 == 1:
                        nc.scalar.dma_start(
                            od[b, :, j, lo - 1024:hi],
                            xb[:, j, lo - 1024:hi],
                        )

                for j in range(J):
                    for dp in range(4):
                        yield lambda j=j, dp=dp: unit(j, dp)

            def drain(it):
                for f in it:
                    f()

            xts = [xt_pool.tile([P, CH, BLK], BF16, tag="xt", name=f"xt{b}")
                   for b in range(NB)]

            drain(front_units(0))
            for b in range(1, NB):
                fu = list(front_units(b))
                bu = list(back_units(b - 1))
                order = []
                for i, f in enumerate(fu):
                    if i < len(bu):
                        order.append(bu[i])
                    order.append(f)
                drain(order)
            drain(back_units(NB - 1))

    nc.compile()
    return nc


def _get_program():
    if "nc" not in _CACHE:
        _CACHE["nc"] = _build_program()
    return _CACHE["nc"]


def kernel(input, hra_u, **run_kwargs):
    input = np.asarray(input, dtype=np.float32)
    hra_u = np.asarray(hra_u, dtype=np.float32)

    A_pad, UT = _householder_wy(hra_u)
    # pack A_pad [D, 40] so partition p holds A_pad[c*128+p, :] at offset c*40
    a_packed = np.ascontiguousarray(
        A_pad.reshape(CH, P, MPAD).transpose(1, 0, 2).reshape(P, CH * MPAD)
    ).astype(NPBF16)
    ut_b = UT.astype(NPBF16)
    ident = np.eye(P, dtype=NPBF16)

    x_flat = input.reshape(ROWS, D).astype(NPBF16)
    in_maps = [
        {
            "x": x_flat[c * RPC:(c + 1) * RPC],
            "a": a_packed,
            "ut": ut_b,
            "ident": ident,
        }
        for c in range(N_CORES)
    ]

    nc = _get_program()
    res = run_bass_kernel_spmd(nc, in_maps, core_ids=list(range(N_CORES)),
                               **run_kwargs)
    out = np.concatenate([r["out"] for r in res.results], axis=0)
    if run_kwargs:
        kernel.last_results = res
    return out.astype(np.float32).reshape(B, S, D)
